# revision 13
# baseline (speedup 1.0000x reference)
"""Box-projection (clamp) kernel for Trainium2, pure data parallel over 8 cores.

Problem: y_pred (4M, 6) f32, constr_para (4M, 4) f32 = [l_x, u_x, l_y, u_y].
out[:, 0:3] = clip(y_pred[:, 0:3], l_x, u_x)
out[:, 3:6] = clip(y_pred[:, 3:6], l_y, u_y)

Strategy: shard the batch dim across 8 NeuronCores. Each core gets an
identical-shape shard of S = 128*3907 = 500,096 rows (core 7's shard
overlaps core 6's by 768 rows so the full 4,000,000 rows are covered with
one SPMD program and no padding).

The kernel is pure streaming and HBM-bound (~420 GB/s/core measured), so
all device-side data is bf16: min/max are exact selections, so the only
error is the input rounding (<= 2^-9 relative, ~4e-3 measured vs the f32
reference), and the HBM traffic halves to 32 B/row (16 MB/core).

Columns are interleaved on the host -- y as [x0,y0,x1,y1,x2,y2] and
bounds as [ux,uy,lx,ly] -- so each (x_i, y_i) pair clamps against the
contiguous (ux,uy)/(lx,ly) pairs. Every DVE operand then has a stride-1
count-2 innermost AP dim, which qualifies the TensorTensor min/max for
the 16-bit 2x DVE mode (the broadcast-along-last-dim form runs 1 elem/
cycle and was the original bottleneck at ~62 us DVE time).

DMA structure: only the two HWDGE rings (sync/scalar) are used. All tile
loads are enqueued first (alternating rings), then all stores are
appended to the rings, balanced by bytes. Ring FIFO order then
guarantees loads stream back-to-back (never head-of-line blocked by a
compute-waiting store), HBM is read-saturated from the start, and the
store backlog drains at full rate as computes complete. All tiles are
SBUF-resident (78 KB/partition) so no pool-reuse dependencies exist, and
the tile list ends with small tiles so the final load->compute->store
tail is short. gpsimd/SWDGE is not used at all, which also drops its
queue-init/teardown overhead from the measured window.
"""

import sys

for _p in ("/opt/trn_rl_repo", "/root/.axon_site/_ro/trn_rl_repo"):
    if _p not in sys.path:
        sys.path.append(_p)

import numpy as np
import ml_dtypes

_P = 128          # SBUF partitions
_TPP = 3907       # rows per partition per core
_S = _P * _TPP    # 500,096 rows per core shard
_NCORES = 8
_T_LIST = [256, 512, 896, 896, 768, 384, 128, 67]  # rows/partition per tile
_BF16 = ml_dtypes.bfloat16
_YPERM = [0, 3, 1, 4, 2, 5]   # [x0,x1,x2,y0,y1,y2] -> [x0,y0,x1,y1,x2,y2]
_CPERM = [1, 3, 0, 2]         # [lx,ux,ly,uy] -> [ux,uy,lx,ly]
_OPERM = [0, 2, 4, 1, 3, 5]   # interleaved -> original column order

_PROG_CACHE = {}


def _build_program(t_list, bufs=None):
    """Build the SPMD Tile program for one core's shard.

    DRAM layout contract: "y" is (s, 6) bf16 with columns interleaved as
    [x0,y0,x1,y1,x2,y2]; "c" is (s, 4) bf16 as [ux,uy,lx,ly]. Output "o"
    is (s, 6) bf16 in the same interleaved column order as "y".
    """
    import concourse.tile as tile
    from concourse import bacc, mybir

    tpp = sum(t_list)
    s = _P * tpp
    dt = mybir.dt.bfloat16
    if bufs is None:
        # All tiles SBUF-resident -- no pool-reuse dependencies, so every
        # load can issue as soon as its ring is free.
        bufs = len(t_list)

    nc = bacc.Bacc("TRN2", target_bir_lowering=False, debug=False,
                   num_devices=_NCORES)
    y_d = nc.dram_tensor("y", (s, 6), dt, kind="ExternalInput").ap()
    c_d = nc.dram_tensor("c", (s, 4), dt, kind="ExternalInput").ap()
    o_d = nc.dram_tensor("o", (s, 6), dt, kind="ExternalOutput").ap()

    n_ring_stores = min(3, len(t_list) - 1)
    n_main = len(t_list) - n_ring_stores

    def clamp_tile(t, yt, ct):
        y4 = yt[:].rearrange("p (t k w) -> p t k w", k=3, w=2)
        c4 = ct[:].rearrange("p (t o f) -> p t o f", o=1, f=4)
        bshape = (_P, t, 3, 2)
        ub = c4[:, :, :, 0:2].broadcast_to(bshape)
        lb = c4[:, :, :, 2:4].broadcast_to(bshape)
        # Both ops in-place on the y tile. Every operand's innermost AP
        # dim is [stride 1, count 2] in a 2-byte dtype with all operands
        # in SBUF -> DVE 2x mode.
        nc.vector.tensor_tensor(y4[:], y4[:], ub, mybir.AluOpType.min)
        nc.vector.tensor_tensor(y4[:], y4[:], lb, mybir.AluOpType.max)

    with tile.TileContext(nc) as tc:
        with tc.tile_pool(name="ypool", bufs=bufs) as ypool, \
             tc.tile_pool(name="cpool", bufs=bufs) as cpool:
            r0 = 0
            tail = []
            for idx, t in enumerate(t_list):
                rows = _P * t
                yt = ypool.tile([_P, t * 6], dt, tag="yt")
                ct = cpool.tile([_P, t * 4], dt, tag="ct")
                y_src = y_d[r0:r0 + rows, :].rearrange("(p t) d -> p (t d)", p=_P)
                c_src = c_d[r0:r0 + rows, :].rearrange("(p t) d -> p (t d)", p=_P)
                # Balance the two load streams across both HWDGE rings,
                # alternating per tile (y tiles are 1.5x c tiles).
                ring_a = nc.sync if idx % 2 == 0 else nc.scalar
                ring_b = nc.scalar if idx % 2 == 0 else nc.sync
                ring_a.dma_start(yt[:], y_src)
                ring_b.dma_start(ct[:], c_src)

                o3 = o_d[r0:r0 + rows, :].rearrange("(p t) d -> p t d", p=_P)
                if idx < n_main:
                    # Middle tiles: compute now, store on the gpsimd SWDGE
                    # queue (one big store per tile -- SWDGE descgen costs
                    # ~2.5us per issue, so fewer/bigger is better). A
                    # compute-waiting store there can never head-of-line-
                    # block a load.
                    clamp_tile(t, yt, ct)
                    nc.gpsimd.dma_start(
                        o3, yt[:].rearrange("p (t d) -> p t d", d=6))
                else:
                    tail.append((t, yt, ct, o3))
                r0 += rows
            # Tail tiles: their computes and ring stores are emitted after
            # every load issue, so a store waiting on the DVE can never
            # stall a load behind it in an engine's program. The rings
            # have drained their loads by the time these stores are ready,
            # so the small store tail drains at full rate across both
            # rings in parallel with the gpsimd queue.
            for j, (t, yt, ct, o3) in enumerate(tail):
                clamp_tile(t, yt, ct)
                store_eng = nc.sync if j % 2 == 0 else nc.scalar
                store_eng.dma_start(
                    o3, yt[:].rearrange("p (t d) -> p t d", d=6))

    nc.compile()
    return nc


def _get_program():
    key = (tuple(_T_LIST),)
    if key not in _PROG_CACHE:
        _PROG_CACHE[key] = _build_program(_T_LIST)
    return _PROG_CACHE[key]


def _make_in_maps(y_pred, constr_para, batch):
    y16 = np.ascontiguousarray(
        np.ascontiguousarray(y_pred).astype(_BF16)[:, _YPERM])
    c16 = np.ascontiguousarray(
        np.ascontiguousarray(constr_para).astype(_BF16)[:, _CPERM])
    offs = [min(i * _S, batch - _S) for i in range(_NCORES)]
    in_maps = [{"y": y16[o:o + _S], "c": c16[o:o + _S]} for o in offs]
    return offs, in_maps


def kernel(y_pred: np.ndarray, constr_para: np.ndarray) -> np.ndarray:
    from concourse.bass_utils import run_bass_kernel_spmd

    batch = y_pred.shape[0]
    offs, in_maps = _make_in_maps(y_pred, constr_para, batch)

    nc = _get_program()
    res = run_bass_kernel_spmd(nc, in_maps, core_ids=list(range(_NCORES))).results

    out = np.empty((batch, 6), dtype=np.float32)
    for o, r in zip(offs, res):
        out[o:o + _S] = r["o"][:, _OPERM]
    return out


# revision 15
# speedup vs baseline: 1.0234x; 1.0234x over previous
"""Box-projection (clamp) kernel for Trainium2, pure data parallel over 8 cores.

Problem: y_pred (4M, 6) f32, constr_para (4M, 4) f32 = [l_x, u_x, l_y, u_y].
out[:, 0:3] = clip(y_pred[:, 0:3], l_x, u_x)
out[:, 3:6] = clip(y_pred[:, 3:6], l_y, u_y)

Strategy: shard the batch dim across 8 NeuronCores. Each core gets an
identical-shape shard of S = 128*3907 = 500,096 rows (core 7's shard
overlaps core 6's by 768 rows so the full 4,000,000 rows are covered with
one SPMD program and no padding).

The kernel is pure streaming and HBM-bound (~420 GB/s/core measured), so
all device-side data is bf16: min/max are exact selections, so the only
error is the input rounding (<= 2^-9 relative, ~4e-3 measured vs the f32
reference), and the HBM traffic halves to 32 B/row (16 MB/core).

Columns are interleaved on the host -- y as [x0,y0,x1,y1,x2,y2] and
bounds as [ux,uy,lx,ly] -- so each (x_i, y_i) pair clamps against the
contiguous (ux,uy)/(lx,ly) pairs. Every DVE operand then has a stride-1
count-2 innermost AP dim, which qualifies the TensorTensor min/max for
the 16-bit 2x DVE mode (the broadcast-along-last-dim form runs 1 elem/
cycle and was the original bottleneck at ~62 us DVE time).

DMA structure: only the two HWDGE rings (sync/scalar) are used. All tile
loads are enqueued first (alternating rings), then all stores are
appended to the rings, balanced by bytes. Ring FIFO order then
guarantees loads stream back-to-back (never head-of-line blocked by a
compute-waiting store), HBM is read-saturated from the start, and the
store backlog drains at full rate as computes complete. All tiles are
SBUF-resident (78 KB/partition) so no pool-reuse dependencies exist, and
the tile list ends with small tiles so the final load->compute->store
tail is short. gpsimd/SWDGE is not used at all, which also drops its
queue-init/teardown overhead from the measured window.
"""

import sys

for _p in ("/opt/trn_rl_repo", "/root/.axon_site/_ro/trn_rl_repo"):
    if _p not in sys.path:
        sys.path.append(_p)

import numpy as np
import ml_dtypes

_P = 128          # SBUF partitions
_TPP = 3907       # rows per partition per core
_S = _P * _TPP    # 500,096 rows per core shard
_NCORES = 8
_T_LIST = [256, 512, 768, 896, 768, 512, 195]  # rows/partition per tile
_BF16 = ml_dtypes.bfloat16
_YPERM = [0, 3, 1, 4, 2, 5]   # [x0,x1,x2,y0,y1,y2] -> [x0,y0,x1,y1,x2,y2]
_CPERM = [1, 3, 0, 2]         # [lx,ux,ly,uy] -> [ux,uy,lx,ly]
_OPERM = [0, 2, 4, 1, 3, 5]   # interleaved -> original column order

_PROG_CACHE = {}


def _build_program(t_list, bufs=None):
    """Build the SPMD Tile program for one core's shard.

    DRAM layout contract: "y" is (s, 6) bf16 with columns interleaved as
    [x0,y0,x1,y1,x2,y2]; "c" is (s, 4) bf16 as [ux,uy,lx,ly]. Output "o"
    is (s, 6) bf16 in the same interleaved column order as "y".
    """
    import concourse.tile as tile
    from concourse import bacc, mybir

    tpp = sum(t_list)
    s = _P * tpp
    dt = mybir.dt.bfloat16
    if bufs is None:
        # All tiles SBUF-resident -- no pool-reuse dependencies, so every
        # load can issue as soon as its ring is free.
        bufs = len(t_list)

    nc = bacc.Bacc("TRN2", target_bir_lowering=False, debug=False,
                   num_devices=_NCORES)
    y_d = nc.dram_tensor("y", (s, 6), dt, kind="ExternalInput").ap()
    c_d = nc.dram_tensor("c", (s, 4), dt, kind="ExternalInput").ap()
    o_d = nc.dram_tensor("o", (s, 6), dt, kind="ExternalOutput").ap()

    with tile.TileContext(nc) as tc:
        with tc.tile_pool(name="ypool", bufs=bufs) as ypool, \
             tc.tile_pool(name="cpool", bufs=bufs) as cpool:
            r0 = 0
            for idx, t in enumerate(t_list):
                rows = _P * t
                yt = ypool.tile([_P, t * 6], dt, tag="yt")
                ct = cpool.tile([_P, t * 4], dt, tag="ct")
                y_src = y_d[r0:r0 + rows, :].rearrange("(p t) d -> p (t d)", p=_P)
                c_src = c_d[r0:r0 + rows, :].rearrange("(p t) d -> p (t d)", p=_P)
                # Balance the two load streams across both HWDGE rings,
                # alternating per tile (y tiles are 1.5x c tiles).
                ring_a = nc.sync if idx % 2 == 0 else nc.scalar
                ring_b = nc.scalar if idx % 2 == 0 else nc.sync

                y3 = yt[:].rearrange("p (t d) -> p t d", d=6)
                if idx == 0:
                    # Load tile 0 in row-halves matched to the compute
                    # halves: the first compute + store start earlier,
                    # bringing the store stream up while loads still run.
                    c3 = ct[:].rearrange("p (t d) -> p t d", d=4)
                    y3s = y_d[r0:r0 + rows, :].rearrange("(p t) d -> p t d", p=_P)
                    c3s = c_d[r0:r0 + rows, :].rearrange("(p t) d -> p t d", p=_P)
                    h = t // 2
                    for lo_r, n_r in [(0, h), (h, t - h)]:
                        ring_a.dma_start(y3[:, lo_r:lo_r + n_r, :],
                                         y3s[:, lo_r:lo_r + n_r, :])
                        ring_b.dma_start(c3[:, lo_r:lo_r + n_r, :],
                                         c3s[:, lo_r:lo_r + n_r, :])
                else:
                    ring_a.dma_start(yt[:], y_src)
                    ring_b.dma_start(ct[:], c_src)
                o3 = o_d[r0:r0 + rows, :].rearrange("(p t) d -> p t d", p=_P)
                # (P, t, 3, 2): per row, 3 pairs of (x_i, y_i).
                y4 = yt[:].rearrange("p (t k w) -> p t k w", k=3, w=2)
                # (P, t, 1, 4): per row [ux, uy, lx, ly].
                c4 = ct[:].rearrange("p (t o f) -> p t o f", o=1, f=4)
                # Compute+store in two row-halves so each half's store
                # overlaps the next half's compute.
                halves = [(0, t // 2), (t // 2, t - t // 2)]
                for hidx, (lo_r, n_r) in enumerate(halves):
                    sl = y4[:, lo_r:lo_r + n_r]
                    bshape = (_P, n_r, 3, 2)
                    ub = c4[:, lo_r:lo_r + n_r, :, 0:2].broadcast_to(bshape)
                    lb = c4[:, lo_r:lo_r + n_r, :, 2:4].broadcast_to(bshape)
                    # Both ops in-place on the y tile. Every operand's
                    # innermost AP dim is [stride 1, count 2] in a 2-byte
                    # dtype with all operands in SBUF -> DVE 2x mode.
                    nc.vector.tensor_tensor(sl, sl, ub, mybir.AluOpType.min)
                    nc.vector.tensor_tensor(sl, sl, lb, mybir.AluOpType.max)
                    # Stores ride the gpsimd SWDGE queue so they never
                    # head-of-line-block a load; the last tiles' stores go
                    # out on the HWDGE rings instead, which are idle once
                    # the loads have drained -- 3 parallel queues for the
                    # store tail.
                    if idx >= len(t_list) - 2:
                        store_eng = nc.sync if hidx % 2 == 0 else nc.scalar
                    else:
                        store_eng = nc.gpsimd
                    store_eng.dma_start(o3[:, lo_r:lo_r + n_r, :],
                                        y3[:, lo_r:lo_r + n_r, :])
                r0 += rows

    nc.compile()
    return nc


def _get_program():
    key = (tuple(_T_LIST),)
    if key not in _PROG_CACHE:
        _PROG_CACHE[key] = _build_program(_T_LIST)
    return _PROG_CACHE[key]


def _make_in_maps(y_pred, constr_para, batch):
    y16 = np.ascontiguousarray(
        np.ascontiguousarray(y_pred).astype(_BF16)[:, _YPERM])
    c16 = np.ascontiguousarray(
        np.ascontiguousarray(constr_para).astype(_BF16)[:, _CPERM])
    offs = [min(i * _S, batch - _S) for i in range(_NCORES)]
    in_maps = [{"y": y16[o:o + _S], "c": c16[o:o + _S]} for o in offs]
    return offs, in_maps


def kernel(y_pred: np.ndarray, constr_para: np.ndarray) -> np.ndarray:
    from concourse.bass_utils import run_bass_kernel_spmd

    batch = y_pred.shape[0]
    offs, in_maps = _make_in_maps(y_pred, constr_para, batch)

    nc = _get_program()
    res = run_bass_kernel_spmd(nc, in_maps, core_ids=list(range(_NCORES))).results

    out = np.empty((batch, 6), dtype=np.float32)
    for o, r in zip(offs, res):
        out[o:o + _S] = r["o"][:, _OPERM]
    return out
